# revision 1
# baseline (speedup 1.0000x reference)
"""RNN-T Joiner kernel for Trainium2 (Bass/Tile), SPMD over 8 NeuronCores.

Math: logits[b,t,u,v] = (enc@W_enc.T + b_enc + dec@W_dec.T + b_dec) @ W_out.T + b_out
    = A[b,t,v] + C[b,u,v]
where A = enc @ (W_out@W_enc).T  (no bias)
      C = dec @ (W_out@W_dec).T + (b_enc+b_dec)@W_out.T + b_out

The (B,T,U,512)@(512,500) product in the reference (73.7 GFLOP) collapses by
linearity into two small matmuls plus a broadcast add, leaving the kernel
output-bandwidth bound (288 MB of logits).

Sharding: data-parallel over batch B=16 -> 2 per core, no collectives.
All host-side work is layout only (slice / transpose / reshape).
"""

import numpy as np

B, T, U, D, V = 16, 300, 30, 512, 500
NCORES = 8
BL = B // NCORES  # batches per core
P = 128
DC = D // P  # 4 contraction chunks

T_CHUNKS = [(0, 128), (128, 128), (256, 44)]
U_GROUPS = [(u, 3) for u in range(0, U, 3)]
UG_MAX = 3

_CACHE = {}


def _build_program():
    from contextlib import ExitStack

    import concourse.bass as bass
    import concourse.tile as tile
    from concourse import bacc, mybir

    f32 = mybir.dt.float32

    nc = bacc.Bacc("TRN2", target_bir_lowering=False, debug=False)

    enc_t = nc.dram_tensor("enc_t", [D, BL * T], f32, kind="ExternalInput").ap()
    dec_t = nc.dram_tensor("dec_t", [D, BL * U], f32, kind="ExternalInput").ap()
    w_enc = nc.dram_tensor("w_enc", [D, D], f32, kind="ExternalInput").ap()
    w_dec = nc.dram_tensor("w_dec", [D, D], f32, kind="ExternalInput").ap()
    w_out_t = nc.dram_tensor("w_out_t", [D, V], f32, kind="ExternalInput").ap()
    b_enc_c = nc.dram_tensor("b_enc_c", [D, 1], f32, kind="ExternalInput").ap()
    b_dec_c = nc.dram_tensor("b_dec_c", [D, 1], f32, kind="ExternalInput").ap()
    b_out_r = nc.dram_tensor("b_out_r", [1, V], f32, kind="ExternalInput").ap()
    iota_d = nc.dram_tensor("iota_d", [BL * U, P], f32, kind="ExternalInput").ap()
    out = nc.dram_tensor("out", [BL, T, U, V], f32, kind="ExternalOutput").ap()

    with tile.TileContext(nc) as tc:
        with ExitStack() as ctx:
            persist = ctx.enter_context(tc.tile_pool(name="persist", bufs=1))

            def _tile(shape, dtype, name):
                return persist.tile(shape, dtype, name=name, tag=name)

            # ---- persistent SBUF tensors ----
            enc_sb = [_tile([P, BL * T], f32, name=f"enc_sb{i}") for i in range(DC)]
            dec_sb = [_tile([P, BL * U], f32, name=f"dec_sb{i}") for i in range(DC)]
            wenc_sb = [_tile([P, D], f32, name=f"wenc_sb{i}") for i in range(DC)]
            wdec_sb = [_tile([P, D], f32, name=f"wdec_sb{i}") for i in range(DC)]
            woutT_sb = [_tile([P, V], f32, name=f"woutT_sb{i}") for i in range(DC)]
            benc_sb = [_tile([P, 1], f32, name=f"benc_sb{i}") for i in range(DC)]
            bdec_sb = [_tile([P, 1], f32, name=f"bdec_sb{i}") for i in range(DC)]
            bout_sb = _tile([1, V], f32, name="bout_sb")
            ones_sb = _tile([1, P], f32, name="ones_sb")
            wceT_sb = [_tile([P, V], f32, name=f"wceT_sb{i}") for i in range(DC)]
            wcdT_sb = [_tile([P, V], f32, name=f"wcdT_sb{i}") for i in range(DC)]
            a_sb = [_tile([P, V], f32, name=f"a_sb{i}") for i in range(BL * len(T_CHUNKS))]
            c_sb = _tile([BL * U, V], f32, name="c_sb")
            iota_sb = _tile([BL * U, P], f32, name="iota_sb")
            bias_sb = _tile([1, V], f32, name="bias_sb")

            # ---- input DMAs ----
            for i in range(DC):
                sl = slice(i * P, (i + 1) * P)
                nc.sync.dma_start(woutT_sb[i][:], w_out_t[sl, :])
                nc.scalar.dma_start(wdec_sb[i][:], w_dec[sl, :])
            nc.sync.dma_start(iota_sb[:], iota_d[:])
            for i in range(DC):
                sl = slice(i * P, (i + 1) * P)
                nc.scalar.dma_start(dec_sb[i][:], dec_t[sl, :])
                nc.sync.dma_start(wenc_sb[i][:], w_enc[sl, :])
                nc.scalar.dma_start(enc_sb[i][:], enc_t[sl, :])
                nc.sync.dma_start(benc_sb[i][:], b_enc_c[sl, :])
                nc.sync.dma_start(bdec_sb[i][:], b_dec_c[sl, :])
            nc.sync.dma_start(bout_sb[:], b_out_r[:])
            nc.any.memset(ones_sb[:], 1.0)

            # ---- setup compute: fused weights, bias row, A, C ----
            with tc.tile_pool(name="psum_s", bufs=2, space="PSUM") as psum_s:
                # WceT[d,v] = sum_j W_enc[j,d] * W_outT[j,v]; same for WcdT
                for w_sb, wt_sb in ((wdec_sb, wcdT_sb), (wenc_sb, wceT_sb)):
                    for dc in range(DC):
                        ps = psum_s.tile([P, V], f32, tag="ps")
                        for jc in range(DC):
                            nc.tensor.matmul(
                                ps[:],
                                w_sb[jc][:, dc * P : (dc + 1) * P],
                                woutT_sb[jc][:],
                                start=(jc == 0),
                                stop=(jc == DC - 1),
                            )
                        nc.any.tensor_copy(wt_sb[dc][:], ps[:])

                # bias_row = (b_enc + b_dec) @ W_out.T + b_out
                ps_b = psum_s.tile([1, V], f32, tag="ps")
                for k, b_sb in enumerate(benc_sb + bdec_sb):
                    nc.tensor.matmul(
                        ps_b[:],
                        b_sb[:],
                        woutT_sb[k % DC][:],
                        start=(k == 0),
                        stop=(k == 2 * DC - 1),
                    )
                nc.any.tensor_add(bias_sb[:], ps_b[:], bout_sb[:])

                # A chunks: A[n,v] = sum_d encT[d,n] * WceT[d,v]
                for bl in range(BL):
                    for tci, (t0, tn) in enumerate(T_CHUNKS):
                        n0 = bl * T + t0
                        ps = psum_s.tile([P, V], f32, tag="ps")
                        for dc in range(DC):
                            nc.tensor.matmul(
                                ps[:tn, :],
                                enc_sb[dc][:, n0 : n0 + tn],
                                wceT_sb[dc][:],
                                start=(dc == 0),
                                stop=(dc == DC - 1),
                            )
                        a = a_sb[bl * len(T_CHUNKS) + tci]
                        nc.any.tensor_copy(a[:tn, :], ps[:tn, :])

                # C: C[m,v] = sum_d decT[d,m] * WcdT[d,v] + bias_row[v]
                ps_c = psum_s.tile([BL * U, V], f32, tag="ps")
                for dc in range(DC):
                    nc.tensor.matmul(
                        ps_c[:],
                        dec_sb[dc][:],
                        wcdT_sb[dc][:],
                        start=(dc == 0),
                        stop=False,
                    )
                nc.tensor.matmul(
                    ps_c[:],
                    ones_sb[:, : BL * U],
                    bias_sb[:],
                    start=False,
                    stop=True,
                )
                nc.any.tensor_copy(c_sb[:], ps_c[:])

            # ---- main loop: broadcast C rows, add A, stream out ----
            crep_pool = ctx.enter_context(
                tc.tile_pool(name="crep", bufs=2, space="PSUM")
            )
            creps_pool = ctx.enter_context(tc.tile_pool(name="crepsb", bufs=2))
            sel_pool = ctx.enter_context(tc.tile_pool(name="selp", bufs=8))
            out_pool = ctx.enter_context(tc.tile_pool(name="outp", bufs=8))

            # Adds split between DVE and Pool (ACT has no tensor_tensor).
            # Pool cannot read PSUM, so its groups get an ACT copy of the
            # broadcast tile into SBUF first.
            gidx = -1
            for bl in range(BL):
                for u0, un in U_GROUPS:
                    gidx += 1
                    on_pool = gidx % 3 == 1
                    crep = crep_pool.tile([P, UG_MAX, 512], f32, tag="crep")
                    for k in range(un):
                        r = bl * U + u0 + k
                        # one-hot row selector: sel[m, p] = (m == r)
                        sel = sel_pool.tile([BL * U, P], f32, tag="sel")
                        nc.vector.tensor_scalar(
                            sel[:],
                            iota_sb[:],
                            float(r),
                            None,
                            mybir.AluOpType.is_equal,
                        )
                        # crep[:, k, v] = sel.T @ C = C[r, v] on every partition
                        nc.tensor.matmul(
                            crep[:, k, :V],
                            sel[:],
                            c_sb[:],
                            start=True,
                            stop=True,
                        )
                    if on_pool:
                        crep_sb = creps_pool.tile([P, UG_MAX, V], f32, tag="csb")
                        nc.scalar.copy(
                            crep_sb[:, :un, :], crep[:, :un, :V]
                        )
                        src_crep = crep_sb
                    for tci, (t0, tn) in enumerate(T_CHUNKS):
                        a = a_sb[bl * len(T_CHUNKS) + tci]
                        ot = out_pool.tile([P, UG_MAX, V], f32, tag="ot")
                        for k in range(un):
                            if on_pool:
                                nc.gpsimd.tensor_add(
                                    ot[:tn, k, :], a[:tn, :], src_crep[:tn, k, :]
                                )
                            else:
                                nc.vector.tensor_add(
                                    ot[:tn, k, :], a[:tn, :], crep[:tn, k, :V]
                                )
                        dma_eng = nc.sync if (tci % 2 == 0) else nc.scalar
                        dma_eng.dma_start(
                            out[bl, t0 : t0 + tn, u0 : u0 + un, :],
                            ot[:tn, :un, :],
                        )

    nc.compile()
    return nc


def _host_prep(inputs):
    """Per-core input maps. Layout-only host work (slice/transpose/reshape)."""
    enc = np.ascontiguousarray(inputs["encoder_out"], dtype=np.float32)
    dec = np.ascontiguousarray(inputs["decoder_out"], dtype=np.float32)
    w_enc = np.ascontiguousarray(inputs["W_enc"], dtype=np.float32)
    w_dec = np.ascontiguousarray(inputs["W_dec"], dtype=np.float32)
    w_out_t = np.ascontiguousarray(inputs["W_out"].T, dtype=np.float32)
    b_enc_c = np.ascontiguousarray(inputs["b_enc"].reshape(D, 1), dtype=np.float32)
    b_dec_c = np.ascontiguousarray(inputs["b_dec"].reshape(D, 1), dtype=np.float32)
    b_out_r = np.ascontiguousarray(inputs["b_out"].reshape(1, V), dtype=np.float32)
    iota = np.broadcast_to(
        np.arange(BL * U, dtype=np.float32)[:, None], (BL * U, P)
    ).copy()

    in_maps = []
    for c in range(NCORES):
        b0 = c * BL
        enc_t = np.ascontiguousarray(enc[b0 : b0 + BL].reshape(BL * T, D).T)
        dec_t = np.ascontiguousarray(dec[b0 : b0 + BL].reshape(BL * U, D).T)
        in_maps.append(
            {
                "enc_t": enc_t,
                "dec_t": dec_t,
                "w_enc": w_enc,
                "w_dec": w_dec,
                "w_out_t": w_out_t,
                "b_enc_c": b_enc_c,
                "b_dec_c": b_dec_c,
                "b_out_r": b_out_r,
                "iota_d": iota,
            }
        )
    return in_maps


def get_program():
    if "nc" not in _CACHE:
        _CACHE["nc"] = _build_program()
    return _CACHE["nc"]


def kernel(**inputs) -> np.ndarray:
    from concourse.bass_utils import run_bass_kernel_spmd

    nc = get_program()
    in_maps = _host_prep(inputs)
    res = run_bass_kernel_spmd(nc, in_maps, list(range(NCORES)))
    return np.concatenate([r["out"] for r in res.results], axis=0)

